# revision 19
# baseline (speedup 1.0000x reference)
"""Trainium2 Bass kernel for EpsilonNetGM (forward-diffused GMM score network).

Math (per row x of shape [D]):
    m'_k    = sqrt(acp) * means_k
    logit_k = (x . m'_k)/sigma2 + [log w_k - 0.5*||m'_k||^2/sigma2]
    resp    = softmax_k(logit)
    out     = c * (x - resp @ m'),   c = 1/sqrt(sigma2),  sigma2 = 1 - acp

Data-parallel over 8 NeuronCores: x/out sharded on the batch axis.

v19 — PE partition-tiling + engine rebalance (28.2us v17 -> ~27.5us):
 - mm1 runs as 8 chunks/pair at PE col-tile positions {0, 64} (AP base
   partitions are limited to {0,32,64}): the logit blocks land STACKED
   on partitions [64c, 64c+64) of ONE [128,512] PSUM bank, so a single
   exp ACT covers a whole 1024-row pair (687ns vs 8x679ns in v17) and
   the 2 col-groups' matmuls overlap on the PE sub-arrays (~2x).
 - mm2 at row-tile positions {0, 64} with nma/lw replicated on
   partitions 64c+k; 8 groups/pair pack 3-3-2 into a [128,3,512] PSUM
   pair tile (bufs=2) + a 9th dummy group so ONE uniform [q,3,3] recip
   covers every s-column. DVE per pair: 1 recip + 2 broadcast muls.
 - Adds (out = tm + c*x, all-f16 2x): pairs 0-2 on GpSimd/Pool (2.1us
   each, runs concurrent with the DVE mul stream), pair 3 on DVE as two
   [128,512] halves; the final store is column-split so each half
   streams as soon as its half-add lands.
 - The DVE combine stream is the sole pacer (PE is LDWEIGHTS-bound at
   ~107ns/MM and has big slack; HAM clock state is irrelevant). DMA:
   both HWDGE queues (~145 B/ns each) carry everything; per-queue FIFO
   gives priority: consts+pair0 head, xt1 split, then xc0 JUMPS AHEAD
   of xt2/xt3 (pool-add 0 needs it early); xc1/xc3 issue later on the
   Scalar ring so they can't delay the exps; stores last. rc bufs=1
   pins the scheduler: recip(p+1) can't hoist past pair p's muls.
 - Consts ride xt0's head columns (a separate tiny DMA pays 128 ring
   descriptors ~= multi-us completion latency); lw4's f32 rides as 2
   bf16 bit-columns, bitcast back on-chip.
"""

import os
import sys

for _p in ("/opt/trn_rl_repo", "/root/.axon_site/_ro/trn_rl_repo"):
    if os.path.isdir(_p) and _p not in sys.path:
        sys.path.insert(0, _p)

import numpy as np
import ml_dtypes
from contextlib import ExitStack

import concourse.bass as bass
import concourse.bacc as bacc
import concourse.tile as tile
from concourse import mybir
from concourse.bass_utils import run_bass_kernel_spmd

N_CORES = 8
N, K, D = 32768, 25, 128
KP = 64                       # padded K (2 col-tile slots of 64)
# consts ride in xt's head columns: [lw-bits(2) | nma(129) | ms(64) | pad]
HC = 2 + 129 + KP + 1         # head columns (bf16) = 196 (even, for bitcast)
N_PER = N // N_CORES          # 4096 rows per core
PAIR = 1024                   # rows per pair (processing unit)
NP = N_PER // PAIR            # 4 pairs per core

F32 = mybir.dt.float32
F16 = mybir.dt.float16
BF16 = mybir.dt.bfloat16
AF = mybir.ActivationFunctionType
OP = mybir.AluOpType

POOL_ADDS = (0, 1, 2)         # pairs whose final add runs on GpSimd/Pool


def build_program():
    nc = bacc.Bacc("TRN2", debug=False)

    # All consts ride in xt's head columns (a separate tiny DMA pays 128
    # ring descriptors ~= multi-us completion latency): [lw4-as-2-bf16-bit-
    # cols | nma4 | ms64(zero-padded so every partition of the logit bank
    # is written -> exp reads no stale PSUM)].
    xt_d = nc.dram_tensor("xt", [D, HC + N_PER], BF16, kind="ExternalInput").ap()
    xc_d = nc.dram_tensor("xc", [N_PER, D], F16, kind="ExternalInput").ap()
    out_d = nc.dram_tensor("out", [N_PER, D], F16, kind="ExternalOutput").ap()

    with tile.TileContext(nc) as tc, ExitStack() as ctx:
        consts = ctx.enter_context(tc.tile_pool(name="consts", bufs=1))
        xt_p = ctx.enter_context(tc.tile_pool(name="xt", bufs=NP))
        xc_p = ctx.enter_context(tc.tile_pool(name="xc", bufs=NP))
        eta_p = ctx.enter_context(tc.tile_pool(name="eta", bufs=2))
        # bufs=1: recip(p+1) must wait for pair p's muls to consume rc8
        # (keeps the scheduler from hoisting it ahead of them)
        rc_p = ctx.enter_context(tc.tile_pool(name="rc", bufs=1))
        tm_p = ctx.enter_context(tc.tile_pool(name="tm", bufs=3))
        o2_p = ctx.enter_context(tc.tile_pool(name="o2", bufs=3))
        ps_st = ctx.enter_context(tc.tile_pool(name="ps_st", bufs=2, space="PSUM"))
        ps_v = ctx.enter_context(tc.tile_pool(name="ps_v", bufs=2, space="PSUM"))

        # ---- input DMAs -------------------------------------------------
        # Everything rides the two HWDGE queues (per-queue ~145 B/ns; two
        # queues approach the 16-engine cap). Strict per-queue FIFO gives
        # priority ordering: xt stream first (PE pacing), xc behind it
        # (only needed by the adds), stores last. Pair 0 (with consts in
        # its head) and pair 1 are partition-split across both queues so
        # the early sems fire fastest.
        xt0 = xt_p.tile([128, HC + PAIR], BF16, name="xt0")
        nc.sync.dma_start(xt0[:64, :], xt_d[:64, : HC + PAIR])
        nc.scalar.dma_start(xt0[64:, :], xt_d[64:, : HC + PAIR])
        lw4 = xt0[:, 0:2].bitcast(F32)
        nma4 = xt0[:, 2 : 2 + (D + 1)]
        ms = xt0[:, 2 + (D + 1) : 2 + (D + 1) + KP]

        # The DVE combine stream is the sole pacer; the PE has slack, so
        # xt2/xt3 are deprioritized BEHIND xc0 (pool-add 0 needs it early).
        # sync-q:   c0a, xt1a, xc0, xc2, stores 0/2/3a
        # scalar-q: c0b, xt1b, xt2, xt3, xc1, xc3, stores 1/3b
        xts, xcs = {0: xt0[:, HC:]}, {}

        def xt_dma(p, eng):
            xt = xt_p.tile([128, PAIR], BF16, name="xt")
            eng.dma_start(xt, xt_d[:, HC + PAIR * p : HC + PAIR * (p + 1)])
            xts[p] = xt

        def xc_dma(p, eng):
            xc = xc_p.tile([128, PAIR], F16, name="xc")
            eng.dma_start(
                xc.rearrange("q (g d) -> q g d", d=D),
                xc_d[PAIR * p : PAIR * (p + 1), :].rearrange(
                    "(q g) d -> q g d", g=8
                ),
            )
            xcs[p] = xc

        xt1 = xt_p.tile([128, PAIR], BF16, name="xt")
        nc.sync.dma_start(xt1[:64], xt_d[:64, HC + PAIR : HC + 2 * PAIR])
        nc.scalar.dma_start(xt1[64:], xt_d[64:, HC + PAIR : HC + 2 * PAIR])
        xts[1] = xt1
        xc_dma(0, nc.sync)
        xt_dma(2, nc.scalar)
        xc_dma(2, nc.sync)
        xt_dma(3, nc.scalar)
        # xc1/xc3 are emitted later (after exp(0)/exp(1)) so their ring
        # issues can't delay the exps on the Scalar sequencer

        # ---- compute ----------------------------------------------------
        psts, etas, pvs, tms, rcs, o2s = {}, {}, {}, {}, {}, {}

        def mm1(p):
            # 8 chunks: (c, w) -> logits of G-block 4c+w on partitions
            # [64c, 64c+64) cols [128w, 128w+128) of one PSUM bank
            # (AP base partitions are limited to {0, 32, 64}, so 2 slots)
            pst = ps_st.tile([128, 512], F32, name="pst")
            xt = xts[p]
            for w in range(4):
                for c in range(2):
                    g = 4 * c + w
                    nc.tensor.matmul(
                        pst[64 * c : 64 * (c + 1), 128 * w : 128 * (w + 1)],
                        lhsT=ms,
                        rhs=xt[:, 128 * g : 128 * (g + 1)],
                        start=True, stop=True,
                    )
            psts[p] = pst

        def exp(p):
            # one ACT per 1024-row pair
            pst = psts.pop(p)
            eta = eta_p.tile([128, 512], BF16, name="eta")
            nc.scalar.activation(eta, pst, AF.Exp, bias=lw4[:, 0:1], scale=1.0)
            etas[p] = eta

        def mm2(p):
            # 8 groups -> [128, 3, 512] pair tile, packed 3-3-2 at col
            # offsets 129j; col 129j+128 accumulates s/c (ones column)
            eta = etas.pop(p)
            pv = ps_v.tile([128, 3, 512], F32, name="pv", tag="pv")
            for w in range(4):
                for c in range(2):
                    g = 4 * c + w
                    b, j = divmod(g, 3)
                    nc.tensor.matmul(
                        pv[:, b : b + 1, 129 * j : 129 * j + 129],
                        lhsT=eta[64 * c : 64 * c + 25, 128 * w : 128 * (w + 1)],
                        rhs=nma4[64 * c : 64 * c + 25, :],
                        start=True, stop=True,
                    )
            # dummy 9th group (repeat G7) fills bank2 j=2 so the recip
            # can use one uniform [q,3,3] AP (PE is LDW-bound, not the
            # bottleneck; one extra 129-col MM is ~107ns)
            nc.tensor.matmul(
                pv[:, 2:3, 129 * 2 : 129 * 2 + 129],
                lhsT=eta[64 + 0 : 64 + 25, 384:512],
                rhs=nma4[64 : 64 + 25, :],
                start=True, stop=True,
            )
            pvs[p] = pv

        def combine(p):
            pv = pvs[p]
            # rc = c/s for all 8 groups + the dummy 9th: one fast-approx
            # recip over the uniform [q,3,3] s-column lattice
            rc8 = rc_p.tile([128, 9], F32, name="rc8")
            nc.vector.reciprocal_approx_fast(
                out=rc8.rearrange("q (b j) -> q b j", j=3),
                in_=pv[:, :, :387].rearrange("q b (j y) -> q b j y", y=129)[
                    :, :, :, 128:129
                ].rearrange("q b j w -> q b (j w)"),
            )
            rcs[p] = rc8

            # tm = V * (c/s); small mul (groups 6-7) first so a hoisted
            # next-pair recip can't stall it behind the big one
            tm = tm_p.tile([128, PAIR], F16, name="tm")
            pvv2 = pv[:, 2:3, :258].rearrange("q b (j y) -> q b j y", y=129)[
                :, :, :, 0:128
            ]
            rcv2 = rc8[:, 6:8].rearrange("q (b j w) -> q b j w", j=2, w=1)
            pv_bc2, rc_bc2 = bass.broadcast_tensor_aps(pvv2, rcv2)
            nc.vector.tensor_mul(
                tm[:, 768:].rearrange("q (b j y) -> q b j y", b=1, j=2),
                pv_bc2, rc_bc2,
            )
            pvv = pv[:, 0:2, :387].rearrange("q b (j y) -> q b j y", y=129)[
                :, :, :, 0:128
            ]
            rcv = rc8[:, 0:6].rearrange("q (b j w) -> q b j w", j=3, w=1)
            pv_bc, rc_bc = bass.broadcast_tensor_aps(pvv, rcv)
            nc.vector.tensor_mul(
                tm[:, 0:768].rearrange("q (b j y) -> q b j y", b=2, j=3), pv_bc, rc_bc
            )
            tms[p] = tm

        def add(p):
            tm = tms.pop(p)
            xc = xcs.pop(p)
            o2 = o2_p.tile([128, PAIR], F16, name="o2")
            if p in POOL_ADDS:
                nc.gpsimd.tensor_add(o2, tm, xc)
            else:
                # two half-pair adds: finer deps + the [128,512] shape
                # reliably hits the DVE 16-bit 2x mode
                for h in range(2):
                    sl = slice(512 * h, 512 * (h + 1))
                    nc.vector.tensor_add(o2[:, sl], tm[:, sl], xc[:, sl])
            o2s[p] = o2

        def store(p, split=False):
            o2 = o2s.pop(p)
            src = o2.rearrange("q (g d) -> q g d", d=D)
            dst = out_d[PAIR * p : PAIR * (p + 1), :].rearrange(
                "(q g) d -> q g d", g=8
            )
            if split:
                # column halves: each gated only on its own half-add, so
                # the first store streams while the last add still runs
                nc.sync.dma_start(dst[:, 0:4, :], src[:, 0:4, :])
                nc.scalar.dma_start(dst[:, 4:8, :], src[:, 4:8, :])
            else:
                eng = nc.scalar if p == 1 else nc.sync
                eng.dma_start(dst, src)

        # pipeline: PE runs mm1 one pair ahead of mm2
        mm1(0)
        exp(0)
        for p in range(NP):
            if p + 1 < NP:
                mm1(p + 1)
            mm2(p)
            if p + 1 < NP:
                exp(p + 1)
            if p == 0:
                xc_dma(1, nc.scalar)
            elif p == 1:
                xc_dma(3, nc.scalar)
            combine(p)
            add(p)
            store(p, split=(p == NP - 1))

    nc.compile()
    return nc


def _host_constants(means, weights, alphas_cumprod, t):
    acp = float(np.asarray(alphas_cumprod, dtype=np.float64)[int(t)])
    sigma2 = 1.0 - acp
    c = 1.0 / np.sqrt(sigma2)
    mprime = np.sqrt(acp) * np.asarray(means, dtype=np.float64)      # [K, D]

    ms = np.zeros((D, KP), dtype=np.float32)
    ms[:, :K] = (mprime / sigma2).T.astype(np.float32)               # [D, KP]
    ms = ms.astype(ml_dtypes.bfloat16)

    # Scales folded into constants: E' = E/c (via -ln c in the bias) and
    # nma = [-c*m' | 1], so the ones column accumulates s/c and
    # tm = (E'@nma) * (c/s) = -(E@m')*c/s; out = tm + c*x.
    logw = np.log(np.asarray(weights, dtype=np.float64))
    lwk = (logw - 0.5 * np.sum(mprime * mprime, axis=1) / sigma2 - np.log(c))
    lw4 = np.zeros((128, 1), dtype=np.float32)
    nma4 = np.zeros((128, D + 1), dtype=np.float32)
    for cc in range(2):
        lw4[64 * cc : 64 * cc + K, 0] = lwk.astype(np.float32)
        nma4[64 * cc : 64 * cc + K, :D] = (-c * mprime).astype(np.float32)
        nma4[64 * cc : 64 * cc + K, D] = 1.0
    nma4 = nma4.astype(ml_dtypes.bfloat16)

    # head block [128, HC] bf16: lw4's f32 bits as 2 bf16 cols, nma4, ms
    head = np.zeros((128, HC), dtype=ml_dtypes.bfloat16)
    head[:, 0:2] = lw4.view(np.uint16).reshape(128, 2).view(ml_dtypes.bfloat16)
    head[:, 2 : 2 + (D + 1)] = nma4
    head[:, 2 + (D + 1) : 2 + (D + 1) + KP] = ms
    return float(c), head


def _host_split_x(x, c):
    # Transposed bf16 copy, columns permuted so col (1024P + 128g + q)
    # holds row (1024P + 8q + g): mm2 stationary slices stay contiguous
    # and each xc/out partition holds 8 consecutive rows (2KB DMA runs).
    v = x.reshape(-1, 128, 8, D)                        # [P, q, g, d]
    xt = v.transpose(3, 0, 2, 1).reshape(D, -1).astype(ml_dtypes.bfloat16)
    xc = (np.float32(c) * x).astype(np.float16)
    return xt, xc


def _build(inputs):
    x = np.ascontiguousarray(np.asarray(inputs["x"], dtype=np.float32))
    assert x.shape == (N, D), x.shape
    c, head = _host_constants(
        inputs["means"], inputs["weights"], inputs["alphas_cumprod"], inputs["t"]
    )

    nc = build_program()
    in_maps = []
    for i in range(N_CORES):
        sl = slice(i * N_PER, (i + 1) * N_PER)
        xt, xc = _host_split_x(x[sl], c)
        xtm = np.ascontiguousarray(np.concatenate([head, xt], axis=1))
        in_maps.append({"xt": xtm, "xc": xc})
    return nc, in_maps


def kernel(x, means, weights, alphas_cumprod, t):
    nc, in_maps = _build({
        "x": x, "means": means, "weights": weights,
        "alphas_cumprod": alphas_cumprod, "t": t,
    })
    res = run_bass_kernel_spmd(nc, in_maps, list(range(N_CORES)))
    out = np.concatenate([res.results[i]["out"] for i in range(N_CORES)], axis=0)
    return out.astype(np.float32, copy=False)


if __name__ == "__main__":
    rng = np.random.default_rng(0)
    x = rng.standard_normal((N, D), dtype=np.float32)
    means = 2.0 * rng.standard_normal((K, D)).astype(np.float32)
    w = rng.uniform(0.1, 1.0, K).astype(np.float32)
    weights = w / w.sum()
    betas = np.linspace(1e-4, 0.02, 1000, dtype=np.float32)
    acp = np.cumprod(1.0 - betas).astype(np.float32)
    out = kernel(x, means, weights, acp, 500)
    print("out", out.shape, out.dtype, out[:2, :4])


# revision 23
# speedup vs baseline: 1.0200x; 1.0200x over previous
"""Trainium2 Bass kernel for EpsilonNetGM (forward-diffused GMM score network).

Math (per row x of shape [D]):
    m'_k    = sqrt(acp) * means_k
    logit_k = (x . m'_k)/sigma2 + [log w_k - 0.5*||m'_k||^2/sigma2]
    resp    = softmax_k(logit)
    out     = c * (x - resp @ m'),   c = 1/sqrt(sigma2),  sigma2 = 1 - acp

Data-parallel over 8 NeuronCores: x/out sharded on the batch axis.

v19 — PE partition-tiling + engine rebalance (28.2us v17 -> ~27.5us):
 - mm1 runs as 8 chunks/pair at PE col-tile positions {0, 64} (AP base
   partitions are limited to {0,32,64}): the logit blocks land STACKED
   on partitions [64c, 64c+64) of ONE [128,512] PSUM bank, so a single
   exp ACT covers a whole 1024-row pair (687ns vs 8x679ns in v17) and
   the 2 col-groups' matmuls overlap on the PE sub-arrays (~2x).
 - mm2 at row-tile positions {0, 64} with nma/lw replicated on
   partitions 64c+k; 8 groups/pair pack 3-3-2 into a [128,3,512] PSUM
   pair tile (bufs=2) + a 9th dummy group so ONE uniform [q,3,3] recip
   covers every s-column. DVE per pair: 1 recip + 2 broadcast muls.
 - Adds (out = tm + c*x, all-f16 2x): pairs 0-2 on GpSimd/Pool (2.1us
   each, runs concurrent with the DVE mul stream), pair 3 on DVE as two
   [128,512] halves; the final store is column-split so each half
   streams as soon as its half-add lands.
 - The DVE combine stream is the sole pacer (PE is LDWEIGHTS-bound at
   ~107ns/MM and has big slack; HAM clock state is irrelevant). DMA:
   both HWDGE queues (~145 B/ns each) carry everything; per-queue FIFO
   gives priority: consts+pair0 head, xt1 split, then xc0 JUMPS AHEAD
   of xt2/xt3 (pool-add 0 needs it early); xc1/xc3 issue later on the
   Scalar ring so they can't delay the exps; stores last. rc bufs=1
   pins the scheduler: recip(p+1) can't hoist past pair p's muls.
 - Consts ride xt0's head columns (a separate tiny DMA pays 128 ring
   descriptors ~= multi-us completion latency); lw4's f32 rides as 2
   bf16 bit-columns, bitcast back on-chip.
"""

import os
import sys

for _p in ("/opt/trn_rl_repo", "/root/.axon_site/_ro/trn_rl_repo"):
    if os.path.isdir(_p) and _p not in sys.path:
        sys.path.insert(0, _p)

import numpy as np
import ml_dtypes
from contextlib import ExitStack

import concourse.bass as bass
import concourse.bacc as bacc
import concourse.tile as tile
from concourse import mybir
from concourse.bass_utils import run_bass_kernel_spmd

N_CORES = 8
N, K, D = 32768, 25, 128
KP = 64                       # padded K (2 col-tile slots of 64)
# consts ride in xt's head columns: [lw-bits(2) | nma(129) | ms(64) | pad]
HC = 2 + 129 + KP + 1         # head columns (bf16) = 196 (even, for bitcast)
N_PER = N // N_CORES          # 4096 rows per core
PAIR = 1024                   # rows per pair (processing unit)
NP = N_PER // PAIR            # 4 pairs per core

F32 = mybir.dt.float32
F16 = mybir.dt.float16
BF16 = mybir.dt.bfloat16
AF = mybir.ActivationFunctionType
OP = mybir.AluOpType

POOL_ADDS = (0, 1, 2)         # pairs whose final add runs on GpSimd/Pool


def build_program():
    nc = bacc.Bacc("TRN2", debug=False)

    # All consts ride in xt's head columns (a separate tiny DMA pays 128
    # ring descriptors ~= multi-us completion latency): [lw4-as-2-bf16-bit-
    # cols | nma4 | ms64(zero-padded so every partition of the logit bank
    # is written -> exp reads no stale PSUM)].
    xt_d = nc.dram_tensor("xt", [D, HC + N_PER], BF16, kind="ExternalInput").ap()
    xc_d = nc.dram_tensor("xc", [N_PER, D], F16, kind="ExternalInput").ap()
    out_d = nc.dram_tensor("out", [N_PER, D], F16, kind="ExternalOutput").ap()

    with tile.TileContext(nc) as tc, ExitStack() as ctx:
        consts = ctx.enter_context(tc.tile_pool(name="consts", bufs=1))
        xt_p = ctx.enter_context(tc.tile_pool(name="xt", bufs=NP))
        xc_p = ctx.enter_context(tc.tile_pool(name="xc", bufs=NP))
        eta_p = ctx.enter_context(tc.tile_pool(name="eta", bufs=2))
        # bufs=1: recip(p+1) must wait for pair p's muls to consume rc8
        # (keeps the scheduler from hoisting it ahead of them)
        rc_p = ctx.enter_context(tc.tile_pool(name="rc", bufs=1))
        tm_p = ctx.enter_context(tc.tile_pool(name="tm", bufs=3))
        o2_p = ctx.enter_context(tc.tile_pool(name="o2", bufs=3))
        ps_st = ctx.enter_context(tc.tile_pool(name="ps_st", bufs=2, space="PSUM"))
        ps_v = ctx.enter_context(tc.tile_pool(name="ps_v", bufs=2, space="PSUM"))

        # ---- input DMAs -------------------------------------------------
        # Everything rides the two HWDGE queues (per-queue ~145 B/ns; two
        # queues approach the 16-engine cap). Strict per-queue FIFO gives
        # priority ordering: xt stream first (PE pacing), xc behind it
        # (only needed by the adds), stores last. Pair 0 (with consts in
        # its head) and pair 1 are partition-split across both queues so
        # the early sems fire fastest.
        xt0 = xt_p.tile([128, HC + PAIR], BF16, name="xt0")
        nc.sync.dma_start(xt0[:64, :], xt_d[:64, : HC + PAIR])
        nc.scalar.dma_start(xt0[64:, :], xt_d[64:, : HC + PAIR])
        lw4 = xt0[:, 0:2].bitcast(F32)
        nma4 = xt0[:, 2 : 2 + (D + 1)]
        ms = xt0[:, 2 + (D + 1) : 2 + (D + 1) + KP]

        # The DVE combine stream is the sole pacer; the PE has slack, so
        # xt2/xt3 are deprioritized BEHIND xc0 (pool-add 0 needs it early).
        # sync-q:   c0a, xt1a, xc0, xc2, stores 0/2/3a
        # scalar-q: c0b, xt1b, xt2, xt3, xc1, xc3, stores 1/3b
        xts, xcs = {0: xt0[:, HC:]}, {}

        def xt_dma(p, eng):
            xt = xt_p.tile([128, PAIR], BF16, name="xt")
            eng.dma_start(xt, xt_d[:, HC + PAIR * p : HC + PAIR * (p + 1)])
            xts[p] = xt

        def xc_dma(p, eng):
            xc = xc_p.tile([128, PAIR], F16, name="xc")
            eng.dma_start(
                xc.rearrange("q (g d) -> q g d", d=D),
                xc_d[PAIR * p : PAIR * (p + 1), :].rearrange(
                    "(q g) d -> q g d", g=8
                ),
            )
            xcs[p] = xc

        xt1 = xt_p.tile([128, PAIR], BF16, name="xt")
        nc.sync.dma_start(xt1[:64], xt_d[:64, HC + PAIR : HC + 2 * PAIR])
        nc.scalar.dma_start(xt1[64:], xt_d[64:, HC + PAIR : HC + 2 * PAIR])
        xts[1] = xt1
        xc_dma(0, nc.sync)
        xt_dma(2, nc.scalar)
        xc_dma(2, nc.sync)
        xt_dma(3, nc.scalar)
        # xc1/xc3 are emitted later (after exp(0)/exp(1)) so their ring
        # issues can't delay the exps on the Scalar sequencer

        # ---- compute ----------------------------------------------------
        psts, etas, pvs, tms, rcs, o2s = {}, {}, {}, {}, {}, {}

        def mm1(p):
            # 8 chunks: (c, w) -> logits of G-block 4c+w on partitions
            # [64c, 64c+64) cols [128w, 128w+128) of one PSUM bank
            # (AP base partitions are limited to {0, 32, 64}, so 2 slots)
            pst = ps_st.tile([128, 512], F32, name="pst")
            xt = xts[p]
            for w in range(4):
                for c in range(2):
                    g = 4 * c + w
                    nc.tensor.matmul(
                        pst[64 * c : 64 * (c + 1), 128 * w : 128 * (w + 1)],
                        lhsT=ms,
                        rhs=xt[:, 128 * g : 128 * (g + 1)],
                        start=True, stop=True,
                    )
            psts[p] = pst

        def exp(p):
            # one ACT per 1024-row pair (column-split variants measured
            # consistently slower -- the extra ACT slot perturbs the ring)
            pst = psts.pop(p)
            eta = eta_p.tile([128, 512], BF16, name="eta")
            nc.scalar.activation(eta, pst, AF.Exp, bias=lw4[:, 0:1], scale=1.0)
            etas[p] = eta

        def mm2(p):
            # 8 groups -> [128, 3, 512] pair tile, packed 3-3-2 at col
            # offsets 129j; col 129j+128 accumulates s/c (ones column)
            eta = etas.pop(p)
            pv = ps_v.tile([128, 3, 512], F32, name="pv", tag="pv")
            for w in range(4):
                for c in range(2):
                    g = 4 * c + w
                    b, j = divmod(g, 3)
                    nc.tensor.matmul(
                        pv[:, b : b + 1, 129 * j : 129 * j + 129],
                        lhsT=eta[64 * c : 64 * c + 25, 128 * w : 128 * (w + 1)],
                        rhs=nma4[64 * c : 64 * c + 25, :],
                        start=True, stop=True,
                    )
            # dummy 9th group (repeat G7) fills bank2 j=2 so the recip
            # can use one uniform [q,3,3] AP (PE is LDW-bound, not the
            # bottleneck; one extra 129-col MM is ~107ns)
            nc.tensor.matmul(
                pv[:, 2:3, 129 * 2 : 129 * 2 + 129],
                lhsT=eta[64 + 0 : 64 + 25, 384:512],
                rhs=nma4[64 : 64 + 25, :],
                start=True, stop=True,
            )
            pvs[p] = pv

        def combine(p):
            pv = pvs[p]
            # rc = c/s for all 8 groups + the dummy 9th: one fast-approx
            # recip over the uniform [q,3,3] s-column lattice
            rc8 = rc_p.tile([128, 9], F32, name="rc8")
            nc.vector.reciprocal_approx_fast(
                out=rc8.rearrange("q (b j) -> q b j", j=3),
                in_=pv[:, :, :387].rearrange("q b (j y) -> q b j y", y=129)[
                    :, :, :, 128:129
                ].rearrange("q b j w -> q b (j w)"),
            )
            rcs[p] = rc8

            # tm = V * (c/s); small mul (groups 6-7) first so a hoisted
            # next-pair recip can't stall it behind the big one
            tm = tm_p.tile([128, PAIR], F16, name="tm")
            pvv2 = pv[:, 2:3, :258].rearrange("q b (j y) -> q b j y", y=129)[
                :, :, :, 0:128
            ]
            rcv2 = rc8[:, 6:8].rearrange("q (b j w) -> q b j w", j=2, w=1)
            pv_bc2, rc_bc2 = bass.broadcast_tensor_aps(pvv2, rcv2)
            nc.vector.tensor_mul(
                tm[:, 768:].rearrange("q (b j y) -> q b j y", b=1, j=2),
                pv_bc2, rc_bc2,
            )
            pvv = pv[:, 0:2, :387].rearrange("q b (j y) -> q b j y", y=129)[
                :, :, :, 0:128
            ]
            rcv = rc8[:, 0:6].rearrange("q (b j w) -> q b j w", j=3, w=1)
            pv_bc, rc_bc = bass.broadcast_tensor_aps(pvv, rcv)
            nc.vector.tensor_mul(
                tm[:, 0:768].rearrange("q (b j y) -> q b j y", b=2, j=3), pv_bc, rc_bc
            )
            tms[p] = tm

        def add(p):
            tm = tms.pop(p)
            xc = xcs.pop(p)
            o2 = o2_p.tile([128, PAIR], F16, name="o2")
            if p in POOL_ADDS:
                nc.gpsimd.tensor_add(o2, tm, xc)
            else:
                # two half-pair adds: finer deps + the [128,512] shape
                # reliably hits the DVE 16-bit 2x mode
                for h in range(2):
                    sl = slice(512 * h, 512 * (h + 1))
                    nc.vector.tensor_add(o2[:, sl], tm[:, sl], xc[:, sl])
            o2s[p] = o2

        def store(p, split=False):
            o2 = o2s.pop(p)
            src = o2.rearrange("q (g d) -> q g d", d=D)
            dst = out_d[PAIR * p : PAIR * (p + 1), :].rearrange(
                "(q g) d -> q g d", g=8
            )
            if split:
                # column halves: each gated only on its own half-add, so
                # the first store streams while the last add still runs
                nc.sync.dma_start(dst[:, 0:4, :], src[:, 0:4, :])
                nc.scalar.dma_start(dst[:, 4:8, :], src[:, 4:8, :])
            else:
                eng = nc.scalar if p == 1 else nc.sync
                eng.dma_start(dst, src)

        # pipeline: PE runs mm1 one pair ahead of mm2
        mm1(0)
        exp(0)
        for p in range(NP):
            if p + 1 < NP:
                mm1(p + 1)
            mm2(p)
            if p + 1 < NP:
                exp(p + 1)
            if p == 0:
                xc_dma(1, nc.scalar)
            elif p == 1:
                xc_dma(3, nc.scalar)
            combine(p)
            add(p)
            store(p, split=(p == NP - 1))

    nc.compile()
    return nc


def _host_constants(means, weights, alphas_cumprod, t):
    acp = float(np.asarray(alphas_cumprod, dtype=np.float64)[int(t)])
    sigma2 = 1.0 - acp
    c = 1.0 / np.sqrt(sigma2)
    mprime = np.sqrt(acp) * np.asarray(means, dtype=np.float64)      # [K, D]

    ms = np.zeros((D, KP), dtype=np.float32)
    ms[:, :K] = (mprime / sigma2).T.astype(np.float32)               # [D, KP]
    ms = ms.astype(ml_dtypes.bfloat16)

    # Scales folded into constants: E' = E/c (via -ln c in the bias) and
    # nma = [-c*m' | 1], so the ones column accumulates s/c and
    # tm = (E'@nma) * (c/s) = -(E@m')*c/s; out = tm + c*x.
    logw = np.log(np.asarray(weights, dtype=np.float64))
    lwk = (logw - 0.5 * np.sum(mprime * mprime, axis=1) / sigma2 - np.log(c))
    lw4 = np.zeros((128, 1), dtype=np.float32)
    nma4 = np.zeros((128, D + 1), dtype=np.float32)
    for cc in range(2):
        lw4[64 * cc : 64 * cc + K, 0] = lwk.astype(np.float32)
        nma4[64 * cc : 64 * cc + K, :D] = (-c * mprime).astype(np.float32)
        nma4[64 * cc : 64 * cc + K, D] = 1.0
    nma4 = nma4.astype(ml_dtypes.bfloat16)

    # head block [128, HC] bf16: lw4's f32 bits as 2 bf16 cols, nma4, ms
    head = np.zeros((128, HC), dtype=ml_dtypes.bfloat16)
    head[:, 0:2] = lw4.view(np.uint16).reshape(128, 2).view(ml_dtypes.bfloat16)
    head[:, 2 : 2 + (D + 1)] = nma4
    head[:, 2 + (D + 1) : 2 + (D + 1) + KP] = ms
    return float(c), head


def _host_split_x(x, c):
    # Transposed bf16 copy, columns permuted so col (1024P + 128g + q)
    # holds row (1024P + 8q + g): mm2 stationary slices stay contiguous
    # and each xc/out partition holds 8 consecutive rows (2KB DMA runs).
    v = x.reshape(-1, 128, 8, D)                        # [P, q, g, d]
    xt = v.transpose(3, 0, 2, 1).reshape(D, -1).astype(ml_dtypes.bfloat16)
    xc = (np.float32(c) * x).astype(np.float16)
    return xt, xc


def _build(inputs):
    x = np.ascontiguousarray(np.asarray(inputs["x"], dtype=np.float32))
    assert x.shape == (N, D), x.shape
    c, head = _host_constants(
        inputs["means"], inputs["weights"], inputs["alphas_cumprod"], inputs["t"]
    )

    nc = build_program()
    in_maps = []
    for i in range(N_CORES):
        sl = slice(i * N_PER, (i + 1) * N_PER)
        xt, xc = _host_split_x(x[sl], c)
        xtm = np.ascontiguousarray(np.concatenate([head, xt], axis=1))
        in_maps.append({"xt": xtm, "xc": xc})
    return nc, in_maps


def kernel(x, means, weights, alphas_cumprod, t):
    nc, in_maps = _build({
        "x": x, "means": means, "weights": weights,
        "alphas_cumprod": alphas_cumprod, "t": t,
    })
    res = run_bass_kernel_spmd(nc, in_maps, list(range(N_CORES)))
    out = np.concatenate([res.results[i]["out"] for i in range(N_CORES)], axis=0)
    return out.astype(np.float32, copy=False)


if __name__ == "__main__":
    rng = np.random.default_rng(0)
    x = rng.standard_normal((N, D), dtype=np.float32)
    means = 2.0 * rng.standard_normal((K, D)).astype(np.float32)
    w = rng.uniform(0.1, 1.0, K).astype(np.float32)
    weights = w / w.sum()
    betas = np.linspace(1e-4, 0.02, 1000, dtype=np.float32)
    acp = np.cumprod(1.0 - betas).astype(np.float32)
    out = kernel(x, means, weights, acp, 500)
    print("out", out.shape, out.dtype, out[:2, :4])
